# revision 14
# baseline (speedup 1.0000x reference)
"""Trainium2 Bass kernel for DirectionalConvLayer.

Problem: 4 directional 3-tap convs over [256, 256, 15, 15] fp32 images, one
input per direction (horizontal / vertical / main-diagonal / anti-diagonal
taps), shared weight [256, 256, 3] and bias [256].

Strategy: every direction is a 1-D 3-tap conv along its set of lines
(rows / columns / diagonals / anti-diagonals) with a dense 256x256 channel
mix per tap. On the host, ALL lines of ALL four inputs are packed
back-to-back (no separators) into one flat stream, split across 8 cores at
line boundaries. CORE_COLS=28800 packs with ZERO waste: each core gets
exactly half a direction (128 images x 225 px). The device kernel is
direction-agnostic: a pure 3-tap conv along the flat axis — accumulating
matmuls against +/-1-shifted views of the stream, contraction over C_in in
two 128-chunks. The conv contaminates the two outputs at every line
junction with one known term each; the host subtracts those (two batched
matmuls) during unpacking.

Transfers and matmul operands are float16 (10-bit mantissa, ~ the tensor
engine's own fp32r precision) at full 1-cycle/row PE rate; PSUM accumulates
in fp32. Per core: 60 free-tiles of 480 cols; per tile x 2 cout-chunks:
6 accumulating matmuls (3 taps x 2 cin-chunks), then a vector-engine
PSUM->SBUF cast and DMA out. Bias is added on the host during unpacking.

Head design (trace-driven). Timing model measured across 5 profiled runs:
engines boot ~7.2 us (NEFF init); HWDGE rings deliver first packets at
~9.0 us (SP) / ~10.0 us (Act — boots ~1 us later, stable); a DMA's
completion semaphore fires ~1.0 us after its last packet (ring completion
pipeline). The HAM grants the 2.4 GHz PE clock (cold: 1.2 GHz) after
~3.5 us of sustained PE activity; the grant tolerates idle gaps up to
~0.5 us, a 1.3-1.9 us gap triggers a 3.4 us 50%-duty clamp, and an idle
PE at the grant-decision moment defers the grant entirely (the stream
then runs at 1.2 GHz for ~4 us — measured, costs ~3 us). Consequences:
(1) stream-start critical bytes are minimized: proa = [wt_o0 | x_k0 |
x_k1 of unit 0] (433 KB) is everything the first PSUM group needs, split
48/80 by partition rows across the Act/SP rings (asymmetric to cancel
the Act ring's later boot) and issued FIRST so no prefetch traffic from
any of the 8 cores' symmetric bursts precedes it; wt_o1 (prob, needed 6
matmuls later) rides right behind. (2) 40 warmup matmuls (gated only on
a gpsimd memset) keep the PE busy from ~7.8 us to ~12.1 us, covering the
measured data-semaphore range 12.0-12.6 us so the handoff gap stays
under the grant's tolerance.
Faster alternatives measured/analyzed and rejected on this gate
(rel < 2e-2):
  - fp8e4 DoubleRow (2 cin-chunks per 256-contraction matmul) runs 1.8x
    the PE rate but rel=4.0e-2; any hi/lo split-operand correction needs
    >= 2 extra passes at a 1.89x pair rate, so every corrected variant is
    slower than fp16.
  - Winograd F(2,3)/F(4,3) with HOST-side transforms cuts PE cols 1.5-2x
    but needs 1.5-2.1x the HBM traffic (6 transform streams per 4 outputs
    in fp16, plus 1.5x m-values out) -> memory-bound at ~140+ us/core
    (358 GB/s per-core DMA ceiling), worse than the fp16 PE roofline.
    On-device output transforms need PSUM-fp32 tensor_tensor which only
    DVE (1.04 ns/elem) + Pool (2 ns/elem) can run -> ~150 us wall.
"""
from contextlib import ExitStack

import numpy as np

import concourse.bass as bass
import concourse.tile as tile
from concourse import mybir
from concourse.bass_utils import run_bass_kernel_spmd

P = 128
FT = 480
CORE_COLS = 28800          # 60*480, = 4*256*225/8 exactly (zero pad waste)
H = W = 15
NCORE = 8
WCOLS = 768                # 3 taps x 2 cin-chunks x 128, per cout-chunk

MM_DT = mybir.dt.float16
MM_NP = np.float16


def _build_lines(d):
    if d == 0:
        return [[(i, j) for j in range(W)] for i in range(H)]
    if d == 1:
        return [[(i, j) for i in range(H)] for j in range(W)]
    if d == 2:
        return [
            [(i, i - k) for i in range(max(0, k), min(H, H + k))]
            for k in range(-(W - 1), W)
        ]
    return [
        [(i, s - i) for i in range(max(0, s - (W - 1)), min(H, s + 1))]
        for s in range(H + W - 1)
    ]


def _build_stream_map():
    """Greedy-pack every (direction, image, line) into NCORE x CORE_COLS,
    back-to-back with NO separators. colmap[d, b, i*W+j] = core * CORE_COLS
    + local_col. At every line-to-line junction the device conv contaminates
    the two adjacent outputs (tap w2 of the left line's last cell reads the
    right line's first cell and vice versa); those two known terms are
    subtracted on the host (`_BOUND_STARTS`). Core slices start at line
    starts, so taps at core edges only read the DRAM zero guards. With
    CORE_COLS=28800 the greedy pack is exact: cores 2d, 2d+1 hold direction
    d's images [0,128) and [128,256)."""
    colmap = np.full((4, 256, H * W), -1, np.int64)
    starts = []
    core, col = 0, 0
    for d in range(4):
        lines = _build_lines(d)
        for b in range(256):
            for ln in lines:
                ll = len(ln)
                if col + ll > CORE_COLS:
                    core += 1
                    col = 0
                    assert core < NCORE, "stream overflow"
                if col > 0:
                    starts.append(core * CORE_COLS + col)
                for i, (r, c) in enumerate(ln):
                    colmap[d, b, r * W + c] = core * CORE_COLS + col + i
                col += ll
    assert (colmap >= 0).all()
    return colmap, np.array(starts, np.int64)


_COLMAP, _BOUND_STARTS = _build_stream_map()


def _split_drain_waits(nc, max_waits=1):
    """Workaround for this walrus build's 'Too many sync wait commands' limit
    (1 sync wait per instruction): hoist excess sem-waits onto nop
    instructions inserted right before the instruction on the same engine.
    Sequential waits on one engine queue are equivalent to multiple waits on
    one instruction."""
    # The Tile exit drain (an InstDrain with many waits, immediately followed
    # by an all-engine barrier) may have its waits distributed across ALL
    # engines — each nop then gates that engine's barrier arrival, and the
    # chains dispatch in parallel instead of serially on one queue. For any
    # other instruction the waits must stay on its own engine.
    rr_engines = [
        mybir.EngineType.SP,
        mybir.EngineType.Pool,
        mybir.EngineType.Activation,
        mybir.EngineType.DVE,
        mybir.EngineType.PE,
    ]
    n = 0
    for fn in nc.m.functions:
        for bb in fn.blocks:
            insts = bb.instructions
            i = 0
            while i < len(insts):
                inst = insts[i]
                si = inst.sync_info
                if si is not None and si.on_wait and len(si.on_wait) > max_waits:
                    is_exit_drain = (
                        type(inst).__name__ == "InstDrain" and len(si.on_wait) > 3
                    )
                    extra = list(si.on_wait)[max_waits:]
                    si.on_wait = list(si.on_wait)[:max_waits]
                    for j, wt in enumerate(extra):
                        eng = rr_engines[j % len(rr_engines)] if is_exit_drain else inst.engine
                        nop = mybir.InstNoOp(
                            name=f"I-waitsplit-{n}",
                            engine=eng,
                            sync_info=mybir.SyncInfo(on_wait=[wt], on_update=[]),
                        )
                        nc.register_instruction(nop)
                        n += 1
                        insts.insert(i, nop)
                        i += 1
                i += 1
    return n


def build_program():
    nc = bass.Bass("TRN2", target_bir_lowering=False, debug=False, num_devices=8)
    xin = nc.dram_tensor(
        "xin", [P, 2, CORE_COLS + 2], MM_DT, kind="ExternalInput"
    ).ap()
    # Critical-path combined tensors. proa = [wt_o0 (768) | x_k0 u0 (482) |
    # x_k1 u0 (482)] is everything the first PSUM group (o=0) needs — 433 KB,
    # split by partition halves across BOTH HWDGE rings so stream-start waits
    # on ~216 KB/ring of clean, prefetch-free traffic. prob = [wt_o1] is only
    # needed 6 matmuls (~1.2 us) later and rides right behind.
    proa = nc.dram_tensor(
        "proa", [P, WCOLS + 2 * (FT + 2)], MM_DT, kind="ExternalInput"
    ).ap()
    prob = nc.dram_tensor(
        "prob", [P, WCOLS], MM_DT, kind="ExternalInput"
    ).ap()
    yout = nc.dram_tensor(
        "yout", [P, 2, CORE_COLS], MM_DT, kind="ExternalOutput"
    ).ap()

    # DMA unit sizes in columns: small prologue units so the PE starts
    # quickly, large middle units for DMA efficiency, small epilogue units
    # to shorten the post-matmul tail.
    units = [480, 480, 960] + [2400] * 10 + [480, 960, 960, 240, 240]
    assert sum(units) == CORE_COLS
    assert all(u % FT == 0 or u == 240 for u in units)

    with tile.TileContext(nc) as tc, ExitStack() as ctx:
        cpool = ctx.enter_context(tc.tile_pool(name="const", bufs=1))
        xpool = ctx.enter_context(tc.tile_pool(name="x", bufs=4))
        ypool = ctx.enter_context(tc.tile_pool(name="y", bufs=4))
        ppool = ctx.enter_context(tc.tile_pool(name="ps", bufs=8, space="PSUM"))

        # PE warmup: dummy matmuls gated only on the gpsimd memset, so the PE
        # is busy through the HAM clock-ramp window while the first DMAs land.
        warm = cpool.tile([P, P], MM_DT)
        nc.gpsimd.memset(warm[:], 0.0)
        # 33 warmup matmuls ~= 3.5 us at the cold 1.2 GHz clock: bridges from
        # PE-ready (~7.4 us) to first-data-ready with NO gap — a long gap
        # there resets the HAM ramp and subsequent matmuls run at half rate.
        wps = ppool.tile([P, P], mybir.dt.float32, tag="ps")
        NWARM = 40
        for i in range(NWARM):
            nc.tensor.matmul(
                wps[:], warm[:], warm[:], start=(i == 0), stop=(i == NWARM - 1)
            )

        # Critical DMAs, first on each ring (scalar=Act, sync=SP), each pro
        # tensor split by partition ranges across both rings. The scalar ring
        # boots ~0.6-1.0 us later than the sync ring (measured, stable), so
        # it gets the smaller share (48 vs 80 rows) to finish together.
        HP = P // 2
        SPL = 48
        ta = cpool.tile([P, WCOLS + 2 * (FT + 2)], MM_DT)
        nc.scalar.dma_start(ta[0:SPL, :], proa[0:SPL, :])
        nc.sync.dma_start(ta[SPL:P, :], proa[SPL:P, :])
        tb = cpool.tile([P, WCOLS], MM_DT)
        nc.scalar.dma_start(tb[0:HP, :], prob[0:HP, :])
        nc.sync.dma_start(tb[HP:P, :], prob[HP:P, :])
        # lhsT for (o, t, k) lives at cols [(t*2+k)*128, ...) of ta (o=0) /
        # tb (o=1); unit-0 rhs for chunk k at cols [768 + k*482 + off, ...).
        wtile = (ta, tb)

        base = 0
        for ui, ucol in enumerate(units):
            if ui == 0:
                def rhs_fn(k, s0, s1):
                    return ta[:, WCOLS + k * (FT + 2) + s0 : WCOLS + k * (FT + 2) + s1]
            elif ui <= 2:
                # one tile per cin-chunk, DMA'd on separate HWDGE rings so
                # their descriptors drain in parallel behind the critical set
                xta = xpool.tile([P, ucol + 2], MM_DT)
                nc.sync.dma_start(xta[:], xin[:, 0, base : base + ucol + 2])
                xtb = xpool.tile([P, ucol + 2], MM_DT)
                nc.scalar.dma_start(xtb[:], xin[:, 1, base : base + ucol + 2])
                xk = (xta, xtb)

                def rhs_fn(k, s0, s1, xk=xk):
                    return xk[k][:, s0:s1]
            else:
                xt = xpool.tile([P, 2, ucol + 2], MM_DT)
                nc.sync.dma_start(xt[:], xin[:, :, base : base + ucol + 2])

                def rhs_fn(k, s0, s1, xt=xt):
                    return xt[:, k, s0:s1]
            yt = ypool.tile([P, 2, ucol], MM_DT)
            for off in range(0, ucol, FT):
                width = min(FT, ucol - off)
                for o in range(2):
                    ps = ppool.tile([P, width], mybir.dt.float32, tag="ps")
                    g = 0
                    for k in range(2):
                        for t in range(3):
                            lhsT = wtile[o][:, (t * 2 + k) * P : (t * 2 + k + 1) * P]
                            rhs = rhs_fn(k, off + t, off + t + width)
                            nc.tensor.matmul(
                                ps[:], lhsT, rhs, start=(g == 0), stop=(g == 5)
                            )
                            g += 1
                    nc.vector.tensor_copy(yt[:, o, off : off + width], ps[:])
                    if ui == len(units) - 1:
                        # last unit: split each final DMA by partition halves
                        # across both HWDGE rings (64 descriptors each) so the
                        # post-matmul tail is halved
                        nc.scalar.dma_start(
                            yout[0:HP, o, base + off : base + off + width],
                            yt[0:HP, o, off : off + width],
                        )
                        nc.sync.dma_start(
                            yout[HP:P, o, base + off : base + off + width],
                            yt[HP:P, o, off : off + width],
                        )
                    elif ui >= len(units) - 2:
                        # tail units: ship every (chunk, half) as soon as its
                        # cast lands so only the last DMAs remain after the
                        # final matmul group
                        nc.scalar.dma_start(
                            yout[:, o, base + off : base + off + width],
                            yt[:, o, off : off + width],
                        )
            if ui < len(units) - 2:
                nc.scalar.dma_start(yout[:, :, base : base + ucol], yt[:])
            base += ucol
    _split_drain_waits(nc)
    return nc


def pack_inputs(xs, weight):
    """xs: list of 4 arrays [256, 256, 15, 15] fp32. in_maps for cores 0-7."""
    # w_dev[p, o2, (t, k, m)] = weight[o2*128+m, k*128+p, t]
    w_dev = np.ascontiguousarray(
        weight.reshape(2, P, 2, P, 3)       # [o2, m, k, p, t]
        .transpose(3, 0, 4, 2, 1)           # [p, o2, t, k, m]
    ).astype(MM_NP).reshape(P, 2, WCOLS)

    C = 256
    xflat = np.zeros((C, NCORE * CORE_COLS), MM_NP)
    for d in range(4):
        xflat[:, _COLMAP[d].reshape(-1)] = (
            xs[d].transpose(1, 0, 2, 3).reshape(C, -1).astype(MM_NP)
        )

    in_maps = []
    for core in range(NCORE):
        seg = xflat[:, core * CORE_COLS : (core + 1) * CORE_COLS]
        xin_np = np.zeros((P, 2, CORE_COLS + 2), MM_NP)
        xin_np[:, 0, 1 : CORE_COLS + 1] = seg[:P]
        xin_np[:, 1, 1 : CORE_COLS + 1] = seg[P:]
        proa_np = np.concatenate(
            [w_dev[:, 0], xin_np[:, 0, : FT + 2], xin_np[:, 1, : FT + 2]], axis=1
        )
        prob_np = w_dev[:, 1]
        in_maps.append(
            {
                "xin": xin_np,
                "proa": np.ascontiguousarray(proa_np),
                "prob": np.ascontiguousarray(prob_np),
            }
        )
    return in_maps, xflat


def unpack_outputs(results, bias, weight, xflat):
    O = 256
    yflat = np.empty((O, NCORE * CORE_COLS), np.float32)
    for core in range(NCORE):
        yo = np.asarray(results[core]["yout"])        # [128, 2, CORE_COLS] fp16
        yflat[:, core * CORE_COLS : (core + 1) * CORE_COLS] = (
            yo.transpose(1, 0, 2).reshape(O, CORE_COLS).astype(np.float32)
        )

    # subtract line-junction contamination (fp16-quantized operands match
    # what the device multiplied, so the residual is only PSUM rounding)
    starts = _BOUND_STARTS
    ends = starts - 1
    w0q = weight[:, :, 0].astype(MM_NP).astype(np.float32)
    w2q = weight[:, :, 2].astype(MM_NP).astype(np.float32)
    xs_q = xflat[:, starts].astype(np.float32)
    xe_q = xflat[:, ends].astype(np.float32)
    yflat[:, ends] -= w2q @ xs_q
    yflat[:, starts] -= w0q @ xe_q
    outs = []
    b = bias[None, :, None].astype(np.float32)
    for d in range(4):
        yd = yflat[:, _COLMAP[d].reshape(-1)].reshape(O, 256, H * W)
        yd = yd.transpose(1, 0, 2) + b
        outs.append(np.ascontiguousarray(yd.reshape(256, 256, H, W)))
    return tuple(outs)


def kernel(x0, x1, x2, x3, weight, bias):
    xs = [np.ascontiguousarray(np.asarray(a, dtype=np.float32)) for a in (x0, x1, x2, x3)]
    weight = np.asarray(weight, dtype=np.float32)
    bias = np.asarray(bias, dtype=np.float32)

    nc = build_program()
    in_maps, xflat = pack_inputs(xs, weight)
    res = run_bass_kernel_spmd(nc, in_maps, list(range(NCORE)))
    return unpack_outputs(res.results, bias, weight, xflat)
